# revision 10
# baseline (speedup 1.0000x reference)
"""Trainium2 Bass kernel for GCFAgg-style block:
    q1 = x@W1.T+b1; q2 = x@W2.T+b2; r = x@WR.T+br
    out = (q1 @ q2.T) @ r        (per batch, no softmax)

Algebraic restructuring (no N x N similarity materialization): with
K = W1^T W2, G = x^T x, sx = colsums(x), the output is
    out = x @ P + 1 v^T
    P = K G WR^T + R1,   R1 = (K sx) bR^T + (W1^T b2)(WR sx + n bR)^T  (rank 2)
    v = (G WR^T)^T (W2^T b1) + host-only terms
R1, v's host terms, K and sx are tiny host-side precomputations; the device
computes G, the small 512^2 chain, and the final projection out = x P + v.

Numerics: G via fp8-e4m3 DoubleRow matmuls (2 K-planes per instruction),
chain and final projection in fp16, f32 PSUM accumulation throughout, fp16
output store upcast on host. Measured ~8.4e-3 max rel err vs the f32
reference (tolerance 2e-2).

Schedule notes: PE warmup matmuls ramp the DVFS clock during NEFF bring-up;
constant loads are bandwidth-staggered behind the fp8 x stream; x^T tiles
(fp16) stream during the chain + projection; copies/adds are spread across
the Vector/Pool/Activation engines to keep PSUM evacuation off the PE
critical path; the v computation hides behind the first projection tiles.

Sharding: batch dim B=8, one batch per NeuronCore (data parallel, 8 cores).

Self-contained: hardcodes shapes from the problem spec
(x: [8, 4096, 512] f32; W*: [512, 512]; b*: [512]).
"""
import sys

sys.path.insert(0, "/opt/trn_rl_repo")

import numpy as np
import ml_dtypes

import concourse.bass as bass  # noqa: F401
import concourse.mybir as mybir
import concourse.tile as tile
from concourse import bacc
from concourse.bass_utils import run_bass_kernel_spmd
from concourse.masks import make_identity
from concourse.tile_rust import add_dep_helper

B = 8          # batch -> one per core
N = 4096       # tokens per batch
D = 512        # model dim
NG = 16        # 256-row DoubleRow accumulation steps for G
NGS = 8        # xg DMA supertiles (2 steps each)
NT = 32        # 128-row tiles for the final projection
NTS = 8        # xat DMA supertiles (4 tiles each)
NOS = 16       # output store supertiles (2 tiles each)
N_CORES = 8

F32 = mybir.dt.float32
F16 = mybir.dt.float16
F8 = mybir.dt.float8e4
DR = mybir.MatmulPerfMode.DoubleRow
ACT_COPY = mybir.ActivationFunctionType.Copy

_built = {}


def _build():
    if "nc" in _built:
        return _built["nc"]

    nc = bacc.Bacc("TRN2", target_bir_lowering=False, debug=False,
                   num_devices=N_CORES)

    # xg[s, p, j, i, d] = fp8(x[s*512 + j*256 + i*128 + p, d])
    xg_d = nc.dram_tensor("xg", (NGS, 128, 2, 2, D), F8, kind="ExternalInput")
    # xat[s, p, c, j] covers lhsT tiles of x^T for 4 row-tiles per supertile
    xat_d = nc.dram_tensor("xat", (NTS, 128, 16, 128), F16,
                           kind="ExternalInput")
    # rhat[p, c, :] = WR^T[c*128 + p, :]; khat[p, c, :] = (W1^T W2)^T[c*128+p]
    rhat_d = nc.dram_tensor("rhat", (128, 4, D), F16, kind="ExternalInput")
    khat_d = nc.dram_tensor("khat", (128, 4, D), F16, kind="ExternalInput")
    r1_d = nc.dram_tensor("r1", (128, 4, D), F16, kind="ExternalInput")
    vhost_d = nc.dram_tensor("vhost", (1, D), F16, kind="ExternalInput")
    # out[s, p, j, d] = out_row(s*256 + j*128 + p)[d]
    out_d = nc.dram_tensor("out", (NOS, 128, 2, D), F16, kind="ExternalOutput")

    with tile.TileContext(nc) as tc:
        with (
            tc.tile_pool(name="xg", bufs=8) as xg_pool,
            tc.tile_pool(name="xat", bufs=8) as xat_pool,
            tc.tile_pool(name="const", bufs=1) as const_pool,
            tc.tile_pool(name="gsb", bufs=1) as g_pool,
            tc.tile_pool(name="chain", bufs=1) as chain_pool,
            tc.tile_pool(name="outsb", bufs=4) as out_pool,
        ):
            def copy_to(i, out_ap, in_ap):
                # PSUM-capable copy engines: DVE (vector) and Activation
                if i % 2 == 0:
                    nc.vector.tensor_copy(out_ap, in_ap)
                else:
                    nc.scalar.activation(out_ap, in_ap, ACT_COPY)

            warm_src = const_pool.tile([128, 128], F16, tag="wsrc")
            nc.vector.memset(warm_src[:], 1.0)
            ident = const_pool.tile([128, 128], F16, tag="ident")
            make_identity(nc, ident[:])
            ones_row = const_pool.tile([1, 128], F16, tag="ones")
            nc.vector.memset(ones_row[:], 1.0)
            rhat_sb = const_pool.tile([128, 4, D], F16, tag="rhat")
            khat_sb = const_pool.tile([128, 4, D], F16, tag="khat")
            r1_sb = const_pool.tile([128, 4, D], F16, tag="r1")
            vhost_sb = const_pool.tile([1, D], F16, tag="vhost")

            # ---- PE warmup: ramp the DVFS clock during DMA bring-up ----
            with tc.tile_pool(name="psW", bufs=1, space="PSUM") as psW_pool:
                ps_w = psW_pool.tile([128, 128], F32, tag="warm")
                for _ in range(22):
                    nc.tensor.matmul(ps_w[:], warm_src[:], warm_src[:],
                                     start=True, stop=True)
                warm_sink = const_pool.tile([128, 128], F16, tag="wsink")
                nc.vector.tensor_copy(warm_sink[:], ps_w[:])

            # ---- phase 1: G = x^T x, fp8 DoubleRow, upper block-triangle ----
            with tc.tile_pool(name="psG", bufs=1, space="PSUM") as psG_pool:
                ps_ga = [psG_pool.tile([128, D - c * 128], F32, tag=f"ga{c}",
                                       name=f"ga{c}") for c in range(4)]
                # per-step [128, 2, 512] fp8 slabs; supertile 0 is split so
                # the very first matmul starts half a transfer earlier
                slabs = []
                for s in range(NGS):
                    if s == 0:
                        for j in range(2):
                            xh = xg_pool.tile([128, 1, 2, D], F8, tag="xg0")
                            nc.scalar.dma_start(xh[:],
                                                xg_d.ap()[0][:, j:j + 1, :, :])
                            slabs.append(xh[:, 0])
                    else:
                        xg_t = xg_pool.tile([128, 2, 2, D], F8, tag="xg")
                        nc.sync.dma_start(xg_t[:], xg_d.ap()[s])
                        slabs.append(xg_t[:, 0])
                        slabs.append(xg_t[:, 1])
                gate_mms = []
                for t in range(NG):
                    for c in range(4):
                        mm = nc.tensor.matmul(
                            ps_ga[c][:],
                            slabs[t][:, :, c * 128:(c + 1) * 128],
                            slabs[t][:, :, c * 128:D],
                            start=(t == 0), stop=(t == NG - 1),
                            perf_mode=DR,
                        )
                        if c == 0:
                            gate_mms.append(mm)

                # constant loads staggered behind the xg stream
                def gate(dma, idx, why):
                    add_dep_helper(dma.ins, gate_mms[idx].ins, reason=why)

                gate(nc.gpsimd.dma_start(vhost_sb[:], vhost_d.ap()[:]), 0,
                     "small consts early")
                for c, gi in enumerate([0, 2, 4, 6]):
                    gate(nc.gpsimd.dma_start(rhat_sb[:, c:c + 1, :],
                                             rhat_d.ap()[:, c:c + 1, :]), gi,
                         "rhat chunk interleaved with xg")
                for c, gi in enumerate([9, 11, 13, 15]):
                    gate(nc.gpsimd.dma_start(khat_sb[:, c:c + 1, :],
                                             khat_d.ap()[:, c:c + 1, :]), gi,
                         "khat chunk interleaved with xg")
                for c in range(4):
                    gate(nc.gpsimd.dma_start(r1_sb[:, c:c + 1, :],
                                             r1_d.ap()[:, c:c + 1, :]),
                         NG - 1, "r1 after G stream")

                # G upper blocks -> SBUF fp16; lower blocks via PE
                # transposes emitted just-in-time between the M1 matmul
                # groups, so the PE never idles on a PSUM-evacuation chain.
                # M1 group order [3,2,1,0]: M1[3] needs only upper blocks.
                g_sb = [g_pool.tile([128, D], F16, tag=f"g{c}", name=f"g{c}")
                        for c in range(4)]
                state = {"i": 0}

                def gcopy(c, col):
                    copy_to(state["i"], g_sb[c][:, col * 128:(col + 1) * 128],
                            ps_ga[c][:, (col - c) * 128:(col - c + 1) * 128])
                    state["i"] += 1

                def transpose_block(c2, c1):
                    ps_tr = psG_pool.tile([128, 128], F16, tag="tr", bufs=2)
                    nc.tensor.transpose(
                        ps_tr[:], g_sb[c1][:, c2 * 128:(c2 + 1) * 128],
                        ident[:])
                    copy_to(state["i"], g_sb[c2][:, c1 * 128:(c1 + 1) * 128],
                            ps_tr[:])
                    state["i"] += 1

                with tc.tile_pool(name="psC", bufs=2, space="PSUM") \
                        as psC_pool:
                    m1_sb = [chain_pool.tile([128, D], F16, tag=f"m1{c}",
                                             name=f"m1{c}") for c in range(4)]

                    def m1_group(g1, g2_order):
                        ps = psC_pool.tile([128, D], F32, tag="chain", bufs=2)
                        for i, g2 in enumerate(g2_order):
                            nc.tensor.matmul(
                                ps[:], g_sb[g2][:, g1 * 128:(g1 + 1) * 128],
                                rhat_sb[:, g2, :],
                                start=(i == 0), stop=(i == 3),
                            )
                        copy_to(g1, m1_sb[g1][:], ps[:])

                    # copies stream column-blocks in the order the M1 groups
                    # consume them; transposes slot between matmul groups
                    for c in range(4):
                        gcopy(c, 3)
                    m1_group(3, [0, 1, 2, 3])
                    transpose_block(3, 2)
                    for c in range(3):
                        gcopy(c, 2)
                    m1_group(2, [0, 1, 2, 3])
                    transpose_block(2, 1)
                    transpose_block(3, 1)
                    for c in range(2):
                        gcopy(c, 1)
                    m1_group(1, [0, 1, 2, 3])
                    transpose_block(1, 0)
                    transpose_block(2, 0)
                    transpose_block(3, 0)
                    gcopy(0, 0)
                    m1_group(0, [0, 1, 2, 3])

                    # v is fully host-computed: broadcast the row across
                    # partitions; its copy overlaps the P stage
                    ps_v = psC_pool.tile([128, D], F32, tag="chain", bufs=2)
                    nc.tensor.matmul(ps_v[:], ones_row[0:1, :],
                                     vhost_sb[0:1, :], start=True, stop=True)
                    v_sb = const_pool.tile([128, D], F32, tag="vsb")
                    nc.scalar.activation(v_sb[:], ps_v[:], ACT_COPY)

                    p_sb = [chain_pool.tile([128, D], F16, tag=f"p{c}",
                                            name=f"p{c}") for c in range(4)]
                    for g1 in range(4):
                        ps = psC_pool.tile([128, D], F32, tag="chain", bufs=2)
                        for i, g2 in enumerate([3, 2, 1, 0]):
                            nc.tensor.matmul(
                                ps[:], khat_sb[:, g2, g1 * 128:(g1 + 1) * 128],
                                m1_sb[g2][:],
                                start=(i == 0), stop=(i == 3),
                            )
                        # fused rank-2 host correction: P = K M1 + R1
                        nc.vector.tensor_add(p_sb[g1][:], ps[:],
                                             r1_sb[:, g1, :])

            # ---- phase 3: out = x @ P + v; the v row/broadcast hides
            # behind the first projection supertile's matmuls ----
            with tc.tile_pool(name="psO", bufs=1, space="PSUM") as psO_pool:
                for s in range(NOS):
                    ot2 = out_pool.tile([128, 2, D], F16, tag="ot")
                    pss = []
                    for j in range(2):
                        t = 2 * s + j
                        if t % 4 == 0:
                            xat_t = xat_pool.tile([128, 16, 128], F16,
                                                  tag="xat")
                            xdma = nc.scalar.dma_start(xat_t[:],
                                                       xat_d.ap()[t // 4])
                            add_dep_helper(xdma.ins, gate_mms[NG - 1].ins,
                                           reason="xat after G stream")
                        ps = psO_pool.tile([128, D], F32, tag="out", bufs=6)
                        for c in range(4):
                            nc.tensor.matmul(
                                ps[:], xat_t[:, (t % 4) * 4 + c, :],
                                p_sb[c][:],
                                start=(c == 0), stop=(c == 3),
                            )
                        pss.append(ps)
                    for j in range(2):
                        nc.vector.tensor_add(ot2[:, j, :], pss[j][:], v_sb[:])
                        if s == NOS - 1:
                            # split the last store: each half leaves as soon
                            # as its add drains, shortening the tail
                            eng = nc.sync if j == 0 else nc.scalar
                            eng.dma_start(out_d.ap()[s][:, j:j + 1, :],
                                          ot2[:, j:j + 1, :])
                    if s < NOS - 1:
                        eng = nc.sync if s % 2 == 0 else nc.scalar
                        eng.dma_start(out_d.ap()[s], ot2[:])

    nc.compile()
    _built["nc"] = nc
    return nc


def _prep_host(x, Wq1_w, Wq1_b, Wq2_w, Wq2_b, WR_w, WR_b):
    f16, f8 = np.float16, ml_dtypes.float8_e4m3fn
    f64 = np.float64
    W1, b1 = Wq1_w.astype(f64), Wq1_b.astype(f64)
    W2, b2 = Wq2_w.astype(f64), Wq2_b.astype(f64)
    WR, bR = WR_w.astype(f64), WR_b.astype(f64)

    K = W1.T @ W2                                 # [512, 512]
    u = W2.T @ b1                                 # [512]
    sx = x.sum(axis=1, dtype=f64)                 # [B, 512]

    # xg[b, s, p, j, i, d] = fp8(x[b, s*512 + j*256 + i*128 + p, d])
    x8 = x.astype(f8)
    xg = np.ascontiguousarray(
        x8.reshape(B, NGS, 2, 2, 128, D).transpose(0, 1, 4, 2, 3, 5))
    xat = np.ascontiguousarray(
        x.transpose(0, 2, 1)                      # [B, 512, 4096]
         .reshape(B, 4, 128, NT, 128)             # [b, c, p, t, j]
         .transpose(0, 3, 2, 1, 4)                # [b, t, p, c, j]
         .reshape(B, NTS, 4, 128, 4, 128)         # [b, s, tj, p, c, j]
         .transpose(0, 1, 3, 2, 4, 5)             # [b, s, p, tj, c, j]
         .reshape(B, NTS, 128, 16, 128)
         .astype(f16))

    def chunked(a):   # [512, 512] -> [128, 4, 512]
        return np.ascontiguousarray(
            a.reshape(4, 128, D).transpose(1, 0, 2)).astype(f16)

    rhat = chunked(WR.T)
    khat = chunked(K.T)
    r1 = np.zeros((B, 128, 4, D), f16)
    vhost = np.zeros((B, 1, D), f16)
    for b in range(B):
        U = np.stack([K @ sx[b], W1.T @ b2], axis=1)             # [512, 2]
        V = np.stack([bR, WR @ sx[b] + float(N) * bR], axis=0)   # [2, 512]
        r1[b] = chunked(U @ V)
        # v = WR (G u) + host terms;  G u = x^T (x u) is a cheap matvec chain
        xb = x[b].astype(f64)
        gu = xb.T @ (xb @ u)
        vhost[b, 0] = (WR @ gu + (b1 @ W2 @ sx[b]) * bR
                       + (b1 @ b2) * (WR @ sx[b])
                       + float(N) * (b1 @ b2) * bR).astype(f16)
    return xg, xat, rhat, khat, r1, vhost


def kernel(x, Wq1_w, Wq1_b, Wq2_w, Wq2_b, WR_w, WR_b):
    x = np.asarray(x, dtype=np.float32)
    args = [np.asarray(a, dtype=np.float32)
            for a in (Wq1_w, Wq1_b, Wq2_w, Wq2_b, WR_w, WR_b)]
    xg, xat, rhat, khat, r1, vhost = _prep_host(x, *args)

    nc = _build()
    in_maps = [
        {"xg": xg[b], "xat": xat[b], "rhat": rhat, "khat": khat,
         "r1": r1[b], "vhost": vhost[b]}
        for b in range(B)
    ]
    # the axon-tunneled device occasionally starts in a wedged state
    # (NRT_EXEC_UNIT_UNRECOVERABLE) and recovers on the next attempt
    last_err = None
    for attempt in range(3):
        try:
            res = run_bass_kernel_spmd(nc, in_maps, core_ids=list(range(N_CORES)))
            break
        except Exception as e:  # noqa: BLE001
            last_err = e
            import time as _time
            _time.sleep(2.0)
            try:
                import jax
                jax.clear_caches()
            except Exception:
                pass
    else:
        raise last_err

    out = np.empty((B, N, D), np.float32)
    for b in range(B):
        ob = res.results[b]["out"].astype(np.float32)   # [16, 128, 2, 512]
        out[b] = ob.transpose(0, 2, 1, 3).reshape(N, D)
    return out


# revision 12
# speedup vs baseline: 1.1068x; 1.1068x over previous
"""Trainium2 Bass kernel for GCFAgg-style block:
    q1 = x@W1.T+b1; q2 = x@W2.T+b2; r = x@WR.T+br
    out = (q1 @ q2.T) @ r        (per batch, no softmax)

Algebraic restructuring (no N x N similarity materialization): with
K = W1^T W2, G = x^T x, sx = colsums(x), the output is
    out = x @ P + 1 v^T
    P = K G WR^T + R1,   R1 = (K sx) bR^T + (W1^T b2)(WR sx + n bR)^T  (rank 2)
    v = (G WR^T)^T (W2^T b1) + host-only terms
R1, v's host terms, K and sx are tiny host-side precomputations; the device
computes G, the small 512^2 chain, and the final projection out = x P + v.

Numerics: G via fp8-e4m3 DoubleRow matmuls (2 K-planes per instruction),
chain and final projection in fp16, f32 PSUM accumulation throughout, fp16
output store upcast on host. Measured ~8.4e-3 max rel err vs the f32
reference (tolerance 2e-2).

Schedule notes: PE warmup matmuls ramp the DVFS clock during NEFF bring-up;
constant loads are bandwidth-staggered behind the fp8 x stream; x^T tiles
(fp16) stream during the chain + projection; PSUM evacuation is spread
across the Vector and Activation engines and interleaved just-in-time with
the chain matmul groups so the PE stays busy end to end.

Sharding: batch dim B=8, one batch per NeuronCore (data parallel, 8 cores).

Self-contained: hardcodes shapes from the problem spec
(x: [8, 4096, 512] f32; W*: [512, 512]; b*: [512]).
"""
import sys

sys.path.insert(0, "/opt/trn_rl_repo")

import numpy as np
import ml_dtypes

import concourse.bass as bass  # noqa: F401
import concourse.mybir as mybir
import concourse.tile as tile
from concourse import bacc
from concourse.bass_utils import run_bass_kernel_spmd
from concourse.masks import make_identity
from concourse.tile_rust import add_dep_helper

B = 8          # batch -> one per core
N = 4096       # tokens per batch
D = 512        # model dim
NG = 16        # 256-row DoubleRow accumulation steps for G
NGS = 8        # xg DMA supertiles (2 steps each)
NT = 32        # 128-row tiles for the final projection
NTS = 8        # xat DMA supertiles (4 tiles each)
NOS = 16       # output store supertiles (2 tiles each)
N_CORES = 8

F32 = mybir.dt.float32
F16 = mybir.dt.float16
F8 = mybir.dt.float8e4
DR = mybir.MatmulPerfMode.DoubleRow
ACT_COPY = mybir.ActivationFunctionType.Copy

_built = {}


def _build():
    if "nc" in _built:
        return _built["nc"]

    nc = bacc.Bacc("TRN2", target_bir_lowering=False, debug=False,
                   num_devices=N_CORES)

    # xg[s, p, j, i, d] = fp8(x[s*512 + j*256 + i*128 + p, d])
    xg_d = nc.dram_tensor("xg", (NGS, 128, 2, 2, D), F8, kind="ExternalInput")
    # xat[s, p, c, j] covers lhsT tiles of x^T for 4 row-tiles per supertile
    xat_d = nc.dram_tensor("xat", (NTS, 128, 16, 128), F16,
                           kind="ExternalInput")
    # rhat[p, c, :] = WR^T[c*128 + p, :]; khat[p, c, :] = (W1^T W2)^T[c*128+p]
    rhat_d = nc.dram_tensor("rhat", (128, 4, D), F16, kind="ExternalInput")
    khat_d = nc.dram_tensor("khat", (128, 4, D), F16, kind="ExternalInput")
    r1_d = nc.dram_tensor("r1", (128, 4, D), F16, kind="ExternalInput")
    vhost_d = nc.dram_tensor("vhost", (1, D), F16, kind="ExternalInput")
    # out[s, p, j, d] = out_row(s*256 + j*128 + p)[d]
    out_d = nc.dram_tensor("out", (NOS, 128, 2, D), F16, kind="ExternalOutput")

    with tile.TileContext(nc) as tc:
        with (
            tc.tile_pool(name="xg", bufs=8) as xg_pool,
            tc.tile_pool(name="xat", bufs=8) as xat_pool,
            tc.tile_pool(name="const", bufs=1) as const_pool,
            tc.tile_pool(name="gsb", bufs=1) as g_pool,
            tc.tile_pool(name="chain", bufs=1) as chain_pool,
            tc.tile_pool(name="outsb", bufs=4) as out_pool,
        ):
            def copy_to(i, out_ap, in_ap):
                # PSUM-capable copy engines: DVE (vector) and Activation
                if i % 2 == 0:
                    nc.vector.tensor_copy(out_ap, in_ap)
                else:
                    nc.scalar.activation(out_ap, in_ap, ACT_COPY)

            warm_src = const_pool.tile([128, 128], F16, tag="wsrc")
            nc.vector.memset(warm_src[:], 1.0)
            ident = const_pool.tile([128, 128], F16, tag="ident")
            make_identity(nc, ident[:])
            ones_row = const_pool.tile([1, 128], F16, tag="ones")
            nc.vector.memset(ones_row[:], 1.0)
            rhat_sb = const_pool.tile([128, 4, D], F16, tag="rhat")
            khat_sb = const_pool.tile([128, 4, D], F16, tag="khat")
            r1_sb = const_pool.tile([128, 4, D], F16, tag="r1")
            vhost_sb = const_pool.tile([1, D], F16, tag="vhost")

            # ---- PE warmup: ramp the DVFS clock during DMA bring-up ----
            with tc.tile_pool(name="psW", bufs=1, space="PSUM") as psW_pool:
                ps_w = psW_pool.tile([128, 128], F32, tag="warm")
                for _ in range(20):
                    nc.tensor.matmul(ps_w[:], warm_src[:], warm_src[:],
                                     start=True, stop=True)
                warm_sink = const_pool.tile([128, 128], F16, tag="wsink")
                nc.vector.tensor_copy(warm_sink[:], ps_w[:])

            # ---- phase 1: G = x^T x, fp8 DoubleRow, upper block-triangle ----
            with tc.tile_pool(name="psG", bufs=1, space="PSUM") as psG_pool:
                ps_ga = [psG_pool.tile([128, D - c * 128], F32, tag=f"ga{c}",
                                       name=f"ga{c}") for c in range(4)]
                # per-step [128, 2, 512] fp8 slabs; supertile 0 is split so
                # the very first matmul starts half a transfer earlier
                slabs = []
                for s in range(NGS):
                    if s == 0:
                        for j in range(2):
                            xh = xg_pool.tile([128, 1, 2, D], F8, tag="xg0")
                            nc.scalar.dma_start(xh[:],
                                                xg_d.ap()[0][:, j:j + 1, :, :])
                            slabs.append(xh[:, 0])
                    else:
                        xg_t = xg_pool.tile([128, 2, 2, D], F8, tag="xg")
                        nc.sync.dma_start(xg_t[:], xg_d.ap()[s])
                        slabs.append(xg_t[:, 0])
                        slabs.append(xg_t[:, 1])
                gate_mms = []
                for t in range(NG):
                    for c in range(4):
                        mm = nc.tensor.matmul(
                            ps_ga[c][:],
                            slabs[t][:, :, c * 128:(c + 1) * 128],
                            slabs[t][:, :, c * 128:D],
                            start=(t == 0), stop=(t == NG - 1),
                            perf_mode=DR,
                        )
                        if c == 0:
                            gate_mms.append(mm)

                # constant loads staggered behind the xg stream
                def gate(dma, idx, why):
                    add_dep_helper(dma.ins, gate_mms[idx].ins, reason=why)

                gate(nc.gpsimd.dma_start(vhost_sb[:], vhost_d.ap()[:]), 0,
                     "small consts early")
                for c, gi in enumerate([0, 2, 4, 6]):
                    gate(nc.gpsimd.dma_start(rhat_sb[:, c:c + 1, :],
                                             rhat_d.ap()[:, c:c + 1, :]), gi,
                         "rhat chunk interleaved with xg")
                for c, gi in enumerate([9, 11, 13, 15]):
                    gate(nc.gpsimd.dma_start(khat_sb[:, c:c + 1, :],
                                             khat_d.ap()[:, c:c + 1, :]), gi,
                         "khat chunk interleaved with xg")
                for c in range(4):
                    gate(nc.gpsimd.dma_start(r1_sb[:, c:c + 1, :],
                                             r1_d.ap()[:, c:c + 1, :]),
                         NG - 1, "r1 after G stream")

                # G upper blocks -> SBUF fp16; lower blocks via PE
                # transposes emitted just-in-time between the M1 matmul
                # groups, so the PE never idles on a PSUM-evacuation chain.
                # M1 group order [3,2,1,0]: M1[3] needs only upper blocks.
                g_sb = [g_pool.tile([128, D], F16, tag=f"g{c}", name=f"g{c}")
                        for c in range(4)]
                state = {"i": 0}

                def gcopy(c, col):
                    copy_to(state["i"], g_sb[c][:, col * 128:(col + 1) * 128],
                            ps_ga[c][:, (col - c) * 128:(col - c + 1) * 128])
                    state["i"] += 1

                def transpose_block(c2, c1):
                    ps_tr = psG_pool.tile([128, 128], F16, tag="tr", bufs=2)
                    nc.tensor.transpose(
                        ps_tr[:], g_sb[c1][:, c2 * 128:(c2 + 1) * 128],
                        ident[:])
                    copy_to(state["i"], g_sb[c2][:, c1 * 128:(c1 + 1) * 128],
                            ps_tr[:])
                    state["i"] += 1

                with tc.tile_pool(name="psC", bufs=2, space="PSUM") \
                        as psC_pool:
                    m1_sb = [chain_pool.tile([128, D], F16, tag=f"m1{c}",
                                             name=f"m1{c}") for c in range(4)]

                    def m1_group(g1, g2_order):
                        ps = psC_pool.tile([128, D], F32, tag="chain", bufs=2)
                        for i, g2 in enumerate(g2_order):
                            nc.tensor.matmul(
                                ps[:], g_sb[g2][:, g1 * 128:(g1 + 1) * 128],
                                rhat_sb[:, g2, :],
                                start=(i == 0), stop=(i == 3),
                            )
                        copy_to(g1, m1_sb[g1][:], ps[:])

                    # copies stream column-blocks in the order the M1 groups
                    # consume them; transposes slot between matmul groups
                    for c in range(4):
                        gcopy(c, 3)
                    m1_group(3, [0, 1, 2, 3])
                    transpose_block(3, 2)
                    for c in range(3):
                        gcopy(c, 2)
                    m1_group(2, [0, 1, 2, 3])
                    transpose_block(2, 1)
                    transpose_block(3, 1)
                    for c in range(2):
                        gcopy(c, 1)
                    m1_group(1, [0, 1, 2, 3])
                    transpose_block(1, 0)
                    transpose_block(2, 0)
                    transpose_block(3, 0)
                    gcopy(0, 0)
                    m1_group(0, [0, 1, 2, 3])

                    # v is fully host-computed: broadcast the row across
                    # partitions; its copy overlaps the P stage
                    ps_v = psC_pool.tile([128, D], F32, tag="chain", bufs=2)
                    nc.tensor.matmul(ps_v[:], ones_row[0:1, :],
                                     vhost_sb[0:1, :], start=True, stop=True)
                    v_sb = const_pool.tile([128, D], F32, tag="vsb")
                    nc.scalar.activation(v_sb[:], ps_v[:], ACT_COPY)

                    p_sb = [chain_pool.tile([128, D], F16, tag=f"p{c}",
                                            name=f"p{c}") for c in range(4)]
                    for g1 in range(4):
                        ps = psC_pool.tile([128, D], F32, tag="chain", bufs=2)
                        for i, g2 in enumerate([3, 2, 1, 0]):
                            nc.tensor.matmul(
                                ps[:], khat_sb[:, g2, g1 * 128:(g1 + 1) * 128],
                                m1_sb[g2][:],
                                start=(i == 0), stop=(i == 3),
                            )
                        # fused rank-2 host correction: P = K M1 + R1
                        nc.vector.tensor_add(p_sb[g1][:], ps[:],
                                             r1_sb[:, g1, :])

            # ---- phase 3: out = x @ P + v; the v row/broadcast hides
            # behind the first projection supertile's matmuls ----
            with tc.tile_pool(name="psO", bufs=1, space="PSUM") as psO_pool:
                for s in range(NOS):
                    ot2 = out_pool.tile([128, 2, D], F16, tag="ot")
                    pss = []
                    for j in range(2):
                        t = 2 * s + j
                        if t % 4 == 0:
                            xat_t = xat_pool.tile([128, 16, 128], F16,
                                                  tag="xat")
                            xdma = nc.scalar.dma_start(xat_t[:],
                                                       xat_d.ap()[t // 4])
                            add_dep_helper(xdma.ins, gate_mms[NG - 1].ins,
                                           reason="xat after G stream")
                        ps = psO_pool.tile([128, D], F32, tag="out", bufs=6)
                        for c in range(4):
                            nc.tensor.matmul(
                                ps[:], xat_t[:, (t % 4) * 4 + c, :],
                                p_sb[c][:],
                                start=(c == 0), stop=(c == 3),
                            )
                        pss.append(ps)
                    for j in range(2):
                        nc.vector.tensor_add(ot2[:, j, :], pss[j][:], v_sb[:])
                        if s == NOS - 1:
                            # split the last store: each half leaves as soon
                            # as its add drains, shortening the tail
                            eng = nc.sync if j == 0 else nc.scalar
                            eng.dma_start(out_d.ap()[s][:, j:j + 1, :],
                                          ot2[:, j:j + 1, :])
                    if s < NOS - 1:
                        eng = nc.sync if s % 2 == 0 else nc.scalar
                        eng.dma_start(out_d.ap()[s], ot2[:])

    nc.compile()
    _built["nc"] = nc
    return nc


def _prep_host(x, Wq1_w, Wq1_b, Wq2_w, Wq2_b, WR_w, WR_b):
    f16, f8 = np.float16, ml_dtypes.float8_e4m3fn
    f64 = np.float64
    W1, b1 = Wq1_w.astype(f64), Wq1_b.astype(f64)
    W2, b2 = Wq2_w.astype(f64), Wq2_b.astype(f64)
    WR, bR = WR_w.astype(f64), WR_b.astype(f64)

    K = W1.T @ W2                                 # [512, 512]
    u = W2.T @ b1                                 # [512]
    sx = x.sum(axis=1, dtype=f64)                 # [B, 512]

    # xg[b, s, p, j, i, d] = fp8(x[b, s*512 + j*256 + i*128 + p, d])
    x8 = x.astype(f8)
    xg = np.ascontiguousarray(
        x8.reshape(B, NGS, 2, 2, 128, D).transpose(0, 1, 4, 2, 3, 5))
    xat = np.ascontiguousarray(
        x.transpose(0, 2, 1)                      # [B, 512, 4096]
         .reshape(B, 4, 128, NT, 128)             # [b, c, p, t, j]
         .transpose(0, 3, 2, 1, 4)                # [b, t, p, c, j]
         .reshape(B, NTS, 4, 128, 4, 128)         # [b, s, tj, p, c, j]
         .transpose(0, 1, 3, 2, 4, 5)             # [b, s, p, tj, c, j]
         .reshape(B, NTS, 128, 16, 128)
         .astype(f16))

    def chunked(a):   # [512, 512] -> [128, 4, 512]
        return np.ascontiguousarray(
            a.reshape(4, 128, D).transpose(1, 0, 2)).astype(f16)

    rhat = chunked(WR.T)
    khat = chunked(K.T)
    r1 = np.zeros((B, 128, 4, D), f16)
    vhost = np.zeros((B, 1, D), f16)
    for b in range(B):
        U = np.stack([K @ sx[b], W1.T @ b2], axis=1)             # [512, 2]
        V = np.stack([bR, WR @ sx[b] + float(N) * bR], axis=0)   # [2, 512]
        r1[b] = chunked(U @ V)
        # v = WR (G u) + host terms;  G u = x^T (x u) is a cheap matvec chain
        xb = x[b].astype(f64)
        gu = xb.T @ (xb @ u)
        vhost[b, 0] = (WR @ gu + (b1 @ W2 @ sx[b]) * bR
                       + (b1 @ b2) * (WR @ sx[b])
                       + float(N) * (b1 @ b2) * bR).astype(f16)
    return xg, xat, rhat, khat, r1, vhost


def kernel(x, Wq1_w, Wq1_b, Wq2_w, Wq2_b, WR_w, WR_b):
    x = np.asarray(x, dtype=np.float32)
    args = [np.asarray(a, dtype=np.float32)
            for a in (Wq1_w, Wq1_b, Wq2_w, Wq2_b, WR_w, WR_b)]
    xg, xat, rhat, khat, r1, vhost = _prep_host(x, *args)

    nc = _build()
    in_maps = [
        {"xg": xg[b], "xat": xat[b], "rhat": rhat, "khat": khat,
         "r1": r1[b], "vhost": vhost[b]}
        for b in range(B)
    ]
    # the axon-tunneled device occasionally starts in a wedged state
    # (NRT_EXEC_UNIT_UNRECOVERABLE) and recovers on the next attempt
    last_err = None
    for attempt in range(3):
        try:
            res = run_bass_kernel_spmd(nc, in_maps, core_ids=list(range(N_CORES)))
            break
        except Exception as e:  # noqa: BLE001
            last_err = e
            import time as _time
            _time.sleep(2.0)
            try:
                import jax
                jax.clear_caches()
            except Exception:
                pass
    else:
        raise last_err

    out = np.empty((B, N, D), np.float32)
    for b in range(B):
        ob = res.results[b]["out"].astype(np.float32)   # [16, 128, 2, 512]
        out[b] = ob.transpose(0, 2, 1, 3).reshape(N, D)
    return out


# revision 16
# speedup vs baseline: 1.1619x; 1.0498x over previous
"""Trainium2 Bass kernel for GCFAgg-style block:
    q1 = x@W1.T+b1; q2 = x@W2.T+b2; r = x@WR.T+br
    out = (q1 @ q2.T) @ r        (per batch, no softmax)

Algebraic restructuring (no N x N similarity materialization): with
K = W1^T W2, G = x^T x, sx = colsums(x), the output is
    out = x @ P + 1 v^T
    P = K G WR^T + R1,   R1 = (K sx) bR^T + (W1^T b2)(WR sx + n bR)^T  (rank 2)
    v = (G WR^T)^T (W2^T b1) + host-only terms
R1, v's host terms, K and sx are tiny host-side precomputations; the device
computes G, the small 512^2 chain, and the final projection out = x P + v.

Numerics: G via fp8-e4m3 DoubleRow matmuls (2 K-planes per instruction),
chain and final projection in fp16, f32 PSUM accumulation throughout, fp16
output store upcast on host. Measured ~8.4e-3 max rel err vs the f32
reference (tolerance 2e-2).

Schedule notes: PE warmup matmuls ramp the DVFS clock during NEFF bring-up;
constant loads are bandwidth-staggered behind the fp8 x stream; x^T tiles
(fp16) stream during the chain + projection; PSUM evacuation is spread
across the Vector and Activation engines and interleaved just-in-time with
the chain matmul groups so the PE stays busy end to end.

Sharding: batch dim B=8, one batch per NeuronCore (data parallel, 8 cores).

Self-contained: hardcodes shapes from the problem spec
(x: [8, 4096, 512] f32; W*: [512, 512]; b*: [512]).
"""
import sys

sys.path.insert(0, "/opt/trn_rl_repo")

import numpy as np
import ml_dtypes

import concourse.bass as bass  # noqa: F401
import concourse.mybir as mybir
import concourse.tile as tile
from concourse import bacc
from concourse.bass_utils import run_bass_kernel_spmd
from concourse.masks import make_identity
from concourse.tile_rust import add_dep_helper

B = 8          # batch -> one per core
N = 4096       # tokens per batch
D = 512        # model dim
NG = 16        # 256-row DoubleRow accumulation steps for G
NGS = 8        # xg DMA supertiles (2 steps each)
NT = 32        # 128-row tiles for the final projection
NTS = 8        # xat DMA supertiles (4 tiles each)
NOS = 16       # output store supertiles (2 tiles each)
N_CORES = 8

F32 = mybir.dt.float32
F16 = mybir.dt.float16
F8 = mybir.dt.float8e4
DR = mybir.MatmulPerfMode.DoubleRow
ACT_COPY = mybir.ActivationFunctionType.Copy

_built = {}


def _build():
    if "nc" in _built:
        return _built["nc"]

    nc = bacc.Bacc("TRN2", target_bir_lowering=False, debug=False,
                   num_devices=N_CORES)

    # xg[s, p, j, i, d] = fp8(x[s*512 + j*256 + i*128 + p, d])
    xg_d = nc.dram_tensor("xg", (NGS, 128, 2, 2, D), F8, kind="ExternalInput")
    # xat[s, p, c, j] covers lhsT tiles of x^T for 4 row-tiles per supertile
    xat_d = nc.dram_tensor("xat", (NTS, 128, 16, 128), F16,
                           kind="ExternalInput")
    # rhat[p, c, :] = WR^T[c*128 + p, :]; khat[p, c, :] = (W1^T W2)^T[c*128+p]
    rhat_d = nc.dram_tensor("rhat", (128, 4, D), F16, kind="ExternalInput")
    khat_d = nc.dram_tensor("khat", (128, 4, D), F16, kind="ExternalInput")
    r1_d = nc.dram_tensor("r1", (128, 4, D), F16, kind="ExternalInput")
    vhost_d = nc.dram_tensor("vhost", (1, D), F16, kind="ExternalInput")
    # out[s, p, j, d] = out_row(s*256 + j*128 + p)[d]
    out_d = nc.dram_tensor("out", (NOS, 128, 2, D), F16, kind="ExternalOutput")

    with tile.TileContext(nc) as tc:
        with (
            tc.tile_pool(name="xg", bufs=8) as xg_pool,
            tc.tile_pool(name="xat", bufs=8) as xat_pool,
            tc.tile_pool(name="const", bufs=1) as const_pool,
            tc.tile_pool(name="gsb", bufs=1) as g_pool,
            tc.tile_pool(name="chain", bufs=1) as chain_pool,
            tc.tile_pool(name="outsb", bufs=4) as out_pool,
        ):
            def copy_to(i, out_ap, in_ap):
                # PSUM-capable copy engines: DVE (vector) and Activation
                if i % 2 == 0:
                    nc.vector.tensor_copy(out_ap, in_ap)
                else:
                    nc.scalar.activation(out_ap, in_ap, ACT_COPY)

            warm_src = const_pool.tile([128, 128], F16, tag="wsrc")
            nc.vector.memset(warm_src[:], 1.0)
            ident = const_pool.tile([128, 128], F16, tag="ident")
            make_identity(nc, ident[:])
            ones_row = const_pool.tile([1, 128], F16, tag="ones")
            nc.vector.memset(ones_row[:], 1.0)
            rhat_sb = const_pool.tile([128, 4, D], F16, tag="rhat")
            khat_sb = const_pool.tile([128, 4, D], F16, tag="khat")
            r1_sb = const_pool.tile([128, 4, D], F16, tag="r1")
            vhost_sb = const_pool.tile([1, D], F16, tag="vhost")

            # ---- PE warmup: ramp the DVFS clock during DMA bring-up ----
            with tc.tile_pool(name="psW", bufs=1, space="PSUM") as psW_pool:
                ps_w = psW_pool.tile([128, 128], F32, tag="warm")
                for _ in range(25):
                    nc.tensor.matmul(ps_w[:], warm_src[:], warm_src[:],
                                     start=True, stop=True)
                warm_sink = const_pool.tile([128, 128], F16, tag="wsink")
                nc.vector.tensor_copy(warm_sink[:], ps_w[:])

            # ---- phase 1: G = x^T x, fp8 DoubleRow, upper block-triangle ----
            with tc.tile_pool(name="psG", bufs=1, space="PSUM") as psG_pool:
                ps_ga = [psG_pool.tile([128, D - c * 128], F32, tag=f"ga{c}",
                                       name=f"ga{c}") for c in range(4)]
                # per-step [128, 2, 512] fp8 slabs; supertile 0 is split so
                # the very first matmul starts half a transfer earlier
                slabs = []
                for s in range(NGS):
                    if s == 0:
                        for j, q in ((0, nc.scalar), (1, nc.gpsimd)):
                            xh = xg_pool.tile([128, 1, 2, D], F8, tag="xg0")
                            q.dma_start(xh[:],
                                        xg_d.ap()[0][:, j:j + 1, :, :])
                            slabs.append(xh[:, 0])
                    else:
                        xg_t = xg_pool.tile([128, 2, 2, D], F8, tag="xg")
                        nc.sync.dma_start(xg_t[:], xg_d.ap()[s])
                        slabs.append(xg_t[:, 0])
                        slabs.append(xg_t[:, 1])
                gate_mms = []
                for t in range(NG):
                    for c in range(4):
                        mm = nc.tensor.matmul(
                            ps_ga[c][:],
                            slabs[t][:, :, c * 128:(c + 1) * 128],
                            slabs[t][:, :, c * 128:D],
                            start=(t == 0), stop=(t == NG - 1),
                            perf_mode=DR,
                        )
                        if c == 0:
                            gate_mms.append(mm)

                # DVFS bridge: harmless transposes fill the PSUM-evacuation
                # latency after G so the clock does not drop before M1
                for _ in range(6):
                    ps_fill = psG_pool.tile([128, 128], F16, tag="tr", bufs=2)
                    nc.tensor.transpose(ps_fill[:], warm_src[:], ident[:])

                # constant loads staggered behind the xg stream
                def gate(dma, idx, why):
                    add_dep_helper(dma.ins, gate_mms[idx].ins, reason=why)

                gate(nc.gpsimd.dma_start(vhost_sb[:], vhost_d.ap()[:]), 0,
                     "small consts early")
                for c, gi in enumerate([0, 2, 4, 6]):
                    gate(nc.gpsimd.dma_start(rhat_sb[:, c:c + 1, :],
                                             rhat_d.ap()[:, c:c + 1, :]), gi,
                         "rhat chunk interleaved with xg")
                for c, gi in enumerate([9, 11, 13, 15]):
                    gate(nc.gpsimd.dma_start(khat_sb[:, c:c + 1, :],
                                             khat_d.ap()[:, c:c + 1, :]), gi,
                         "khat chunk interleaved with xg")
                for c in range(4):
                    gate(nc.gpsimd.dma_start(r1_sb[:, c:c + 1, :],
                                             r1_d.ap()[:, c:c + 1, :]),
                         NG - 1, "r1 after G stream")

                # G upper blocks -> SBUF fp16; lower blocks via PE
                # transposes emitted just-in-time between the M1 matmul
                # groups, so the PE never idles on a PSUM-evacuation chain.
                # M1 group order [3,2,1,0]: M1[3] needs only upper blocks.
                g_sb = [g_pool.tile([128, D], F16, tag=f"g{c}", name=f"g{c}")
                        for c in range(4)]
                state = {"i": 0}

                def gcopy(c, col):
                    copy_to(state["i"], g_sb[c][:, col * 128:(col + 1) * 128],
                            ps_ga[c][:, (col - c) * 128:(col - c + 1) * 128])
                    state["i"] += 1

                def transpose_block(c2, c1):
                    ps_tr = psG_pool.tile([128, 128], F16, tag="tr", bufs=2)
                    nc.tensor.transpose(
                        ps_tr[:], g_sb[c1][:, c2 * 128:(c2 + 1) * 128],
                        ident[:])
                    copy_to(state["i"], g_sb[c2][:, c1 * 128:(c1 + 1) * 128],
                            ps_tr[:])
                    state["i"] += 1

                with tc.tile_pool(name="psC", bufs=2, space="PSUM") \
                        as psC_pool:
                    m1_sb = [chain_pool.tile([128, D], F16, tag=f"m1{c}",
                                             name=f"m1{c}") for c in range(4)]

                    def m1_group(g1, g2_order):
                        ps = psC_pool.tile([128, D], F32, tag="chain", bufs=2)
                        for i, g2 in enumerate(g2_order):
                            nc.tensor.matmul(
                                ps[:], g_sb[g2][:, g1 * 128:(g1 + 1) * 128],
                                rhat_sb[:, g2, :],
                                start=(i == 0), stop=(i == 3),
                            )
                        copy_to(g1, m1_sb[g1][:], ps[:])

                    # copies stream column-blocks in the order the M1 groups
                    # consume them; transposes slot between matmul groups
                    for c in range(4):
                        gcopy(c, 3)
                    m1_group(3, [0, 1, 2, 3])
                    transpose_block(3, 2)
                    for c in range(3):
                        gcopy(c, 2)
                    m1_group(2, [0, 1, 2, 3])
                    transpose_block(2, 1)
                    transpose_block(3, 1)
                    for c in range(2):
                        gcopy(c, 1)
                    m1_group(1, [0, 1, 2, 3])
                    transpose_block(1, 0)
                    transpose_block(2, 0)
                    transpose_block(3, 0)
                    gcopy(0, 0)
                    m1_group(0, [0, 1, 2, 3])

                    # v is fully host-computed: broadcast the row across
                    # partitions; its copy overlaps the P stage
                    ps_v = psC_pool.tile([128, D], F32, tag="chain", bufs=2)
                    nc.tensor.matmul(ps_v[:], ones_row[0:1, :],
                                     vhost_sb[0:1, :], start=True, stop=True)
                    v_sb = const_pool.tile([128, D], F32, tag="vsb")
                    nc.scalar.activation(v_sb[:], ps_v[:], ACT_COPY)

                    p_sb = [chain_pool.tile([128, D], F16, tag=f"p{c}",
                                            name=f"p{c}") for c in range(4)]
                    for g1 in range(4):
                        ps = psC_pool.tile([128, D], F32, tag="chain", bufs=2)
                        for i, g2 in enumerate([3, 2, 1, 0]):
                            nc.tensor.matmul(
                                ps[:], khat_sb[:, g2, g1 * 128:(g1 + 1) * 128],
                                m1_sb[g2][:],
                                start=(i == 0), stop=(i == 3),
                            )
                        # fused rank-2 host correction: P = K M1 + R1
                        nc.vector.tensor_add(p_sb[g1][:], ps[:],
                                             r1_sb[:, g1, :])

            # ---- phase 3: out = x @ P + v; the v row/broadcast hides
            # behind the first projection supertile's matmuls ----
            with tc.tile_pool(name="psO", bufs=1, space="PSUM") as psO_pool:
                for s in range(NOS):
                    ot2 = out_pool.tile([128, 2, D], F16, tag="ot")
                    pss = []
                    for j in range(2):
                        t = 2 * s + j
                        if t % 4 == 0:
                            xat_t = xat_pool.tile([128, 16, 128], F16,
                                                  tag="xat")
                            xdma = nc.scalar.dma_start(xat_t[:],
                                                       xat_d.ap()[t // 4])
                            add_dep_helper(xdma.ins, gate_mms[NG - 1].ins,
                                           reason="xat after G stream")
                        ps = psO_pool.tile([128, D], F32, tag="out", bufs=6)
                        for c in range(4):
                            nc.tensor.matmul(
                                ps[:], xat_t[:, (t % 4) * 4 + c, :],
                                p_sb[c][:],
                                start=(c == 0), stop=(c == 3),
                            )
                        pss.append(ps)
                    for j in range(2):
                        nc.vector.tensor_add(ot2[:, j, :], pss[j][:], v_sb[:])
                        if s == NOS - 1:
                            # split the last store: each half leaves as soon
                            # as its add drains, shortening the tail
                            eng = nc.gpsimd if j == 0 else nc.sync
                            eng.dma_start(out_d.ap()[s][:, j:j + 1, :],
                                          ot2[:, j:j + 1, :])
                    if s < NOS - 1:
                        eng = nc.sync if s % 2 == 0 else nc.scalar
                        eng.dma_start(out_d.ap()[s], ot2[:])

    nc.compile()
    _built["nc"] = nc
    return nc


def _prep_host(x, Wq1_w, Wq1_b, Wq2_w, Wq2_b, WR_w, WR_b):
    f16, f8 = np.float16, ml_dtypes.float8_e4m3fn
    f64 = np.float64
    W1, b1 = Wq1_w.astype(f64), Wq1_b.astype(f64)
    W2, b2 = Wq2_w.astype(f64), Wq2_b.astype(f64)
    WR, bR = WR_w.astype(f64), WR_b.astype(f64)

    K = W1.T @ W2                                 # [512, 512]
    u = W2.T @ b1                                 # [512]
    sx = x.sum(axis=1, dtype=f64)                 # [B, 512]

    # xg[b, s, p, j, i, d] = fp8(x[b, s*512 + j*256 + i*128 + p, d])
    x8 = x.astype(f8)
    xg = np.ascontiguousarray(
        x8.reshape(B, NGS, 2, 2, 128, D).transpose(0, 1, 4, 2, 3, 5))
    xat = np.ascontiguousarray(
        x.transpose(0, 2, 1)                      # [B, 512, 4096]
         .reshape(B, 4, 128, NT, 128)             # [b, c, p, t, j]
         .transpose(0, 3, 2, 1, 4)                # [b, t, p, c, j]
         .reshape(B, NTS, 4, 128, 4, 128)         # [b, s, tj, p, c, j]
         .transpose(0, 1, 3, 2, 4, 5)             # [b, s, p, tj, c, j]
         .reshape(B, NTS, 128, 16, 128)
         .astype(f16))

    def chunked(a):   # [512, 512] -> [128, 4, 512]
        return np.ascontiguousarray(
            a.reshape(4, 128, D).transpose(1, 0, 2)).astype(f16)

    rhat = chunked(WR.T)
    khat = chunked(K.T)
    r1 = np.zeros((B, 128, 4, D), f16)
    vhost = np.zeros((B, 1, D), f16)
    for b in range(B):
        U = np.stack([K @ sx[b], W1.T @ b2], axis=1)             # [512, 2]
        V = np.stack([bR, WR @ sx[b] + float(N) * bR], axis=0)   # [2, 512]
        r1[b] = chunked(U @ V)
        # v = WR (G u) + host terms;  G u = x^T (x u) is a cheap matvec chain
        xb = x[b].astype(f64)
        gu = xb.T @ (xb @ u)
        vhost[b, 0] = (WR @ gu + (b1 @ W2 @ sx[b]) * bR
                       + (b1 @ b2) * (WR @ sx[b])
                       + float(N) * (b1 @ b2) * bR).astype(f16)
    return xg, xat, rhat, khat, r1, vhost


def kernel(x, Wq1_w, Wq1_b, Wq2_w, Wq2_b, WR_w, WR_b):
    x = np.asarray(x, dtype=np.float32)
    args = [np.asarray(a, dtype=np.float32)
            for a in (Wq1_w, Wq1_b, Wq2_w, Wq2_b, WR_w, WR_b)]
    xg, xat, rhat, khat, r1, vhost = _prep_host(x, *args)

    nc = _build()
    in_maps = [
        {"xg": xg[b], "xat": xat[b], "rhat": rhat, "khat": khat,
         "r1": r1[b], "vhost": vhost[b]}
        for b in range(B)
    ]
    # the axon-tunneled device occasionally starts in a wedged state
    # (NRT_EXEC_UNIT_UNRECOVERABLE) and recovers on the next attempt
    last_err = None
    for attempt in range(3):
        try:
            res = run_bass_kernel_spmd(nc, in_maps, core_ids=list(range(N_CORES)))
            break
        except Exception as e:  # noqa: BLE001
            last_err = e
            import time as _time
            _time.sleep(2.0)
            try:
                import jax
                jax.clear_caches()
            except Exception:
                pass
    else:
        raise last_err

    out = np.empty((B, N, D), np.float32)
    for b in range(B):
        ob = res.results[b]["out"].astype(np.float32)   # [16, 128, 2, 512]
        out[b] = ob.transpose(0, 2, 1, 3).reshape(N, D)
    return out
